# revision 20
# baseline (speedup 1.0000x reference)
"""GATv2Conv (heads=4, concat=False, self-loops) + GraphNorm on 8 TRN2 NeuronCores.

v6 design notes (on top of v5):
- Prologue restructure: P0 computes xl for table A first, triggers its
  AllGather, then xl for table B + its AllGather, and only then computes
  xr (hidden under the collectives). X transposes are cached in SBUF so
  the xr pass skips the re-transpose. Pair-stream bufs 3->4 so gathers
  ramp while AG-B is still in flight.
- PE/ACT batching: the per-chunk identity-add matmul is batched 2 chunks
  per op (512-col moving limit) into a per-GRP PSUM tile, and the Prelu
  runs once per GRP straight out of that tile. Self-loop path is batched
  per pair (one Prelu/mult/reduce/exp over both windows).
- GraphNorm stats use a transient PSUM tile + SBUF accumulator instead of
  run-long PSUM banks (frees 2 banks for the GRP tiles).
- P4 applies the affine + output DMA in window chunks to overlap the tail.
"""
import os
import sys

sys.path.insert(0, "/opt/trn_rl_repo")

import ml_dtypes
import numpy as np
from concourse import bacc, mybir, tile
from concourse.bass_utils import run_bass_kernel_spmd
from concourse.masks import make_identity

N = 50000
NCORES = 8
NSH = N // NCORES          # 6250 dst nodes per core
SPL = 3200                 # sub-shard split: rows [0:SPL) -> table A
NA = NCORES * SPL          # 25600 rows in table A
NB = NCORES * (NSH - SPL)  # 24400 rows in table B
IN_F = 256
H = 4
C = 64
F = H * C                  # 256
FQ = F + H                 # 260: scatter rhs = [a*p || p]
W = 125                    # dst window size
NW = NSH // W              # 50 windows per core
NP = NW // 2               # 25 window pairs
NEG = 0.2
EPS = 1e-5
MASKVAL = -100.0           # logit bias for padding edges -> exp == 0
CHUNK = 128
GRP = 4                    # chunks fused per DVE op group
SPLT = SPL // CHUNK        # 25 tiles in table-A rows
NTILE = (NSH + 127) // 128  # 49

f32 = mybir.dt.float32
bf16 = mybir.dt.bfloat16
i16 = mybir.dt.int16

LAST_RESULTS = None


def _pack_idx(idx: np.ndarray) -> np.ndarray:
    """[n] int -> [128, n//16] int16 gather-index layout (16-partition wrap,
    replicated for the 8 Q7 cores)."""
    n = idx.shape[0]
    pk = np.zeros((16, n // 16), np.int16)
    pk[np.arange(n) % 16, np.arange(n) // 16] = idx.astype(np.int16)
    return np.tile(pk, (8, 1))


def _prep_edges(src: np.ndarray, dst: np.ndarray):
    """Partition/sort/pad the RANDOM edges (self-loops excluded by caller).
    Pair layout per window pair wp=(2w, 2w+1):
      chunks [lo(2w) | lo(2w+1) | hi(2w) | hi(2w+1)]
    so one gather per table covers both windows. Pad slots keep index 0
    and mask -100.
    Returns (cl, ch, per_core) with IDX/MK/M01/M01T in pair-chunk order."""
    src = src.astype(np.int64)
    dst = dst.astype(np.int64)
    core = dst // NSH
    scid = src // NSH                  # source core
    soff = src % NSH                   # offset within source shard
    is_b = soff >= SPL
    tidx = np.where(is_b, scid * (NSH - SPL) + soff - SPL, scid * SPL + soff)
    per_core_raw = []
    nlo = np.zeros((NCORES, NW), np.int64)
    nhi = np.zeros((NCORES, NW), np.int64)
    for c in range(NCORES):
        m = core == c
        ti = tidx[m]
        hb = is_b[m].astype(np.int64)
        d = dst[m] - c * NSH
        win = d // W
        order = np.lexsort((hb, win))
        ti, d, hb = ti[order], d[order], hb[order]
        key = (d // W) * 2 + hb
        cnt = np.bincount(key, minlength=NW * 2).reshape(NW, 2)
        nlo[c] = cnt[:, 0]
        nhi[c] = cnt[:, 1]
        per_core_raw.append((ti, d, np.cumsum(cnt.reshape(-1))))
    cl = np.ceil(nlo.max(axis=0) / CHUNK).astype(int)
    ch = np.ceil(nhi.max(axis=0) / CHUNK).astype(int)

    dcols = np.arange(W, dtype=np.int64)
    per_core = []
    for c in range(NCORES):
        ti, d, cum = per_core_raw[c]

        def seg(w, half):
            """(tidx, local-d) arrays for (window, half) padded to chunks."""
            k = w * 2 + half
            beg = cum[k - 1] if k > 0 else 0
            end = cum[k]
            t_s, d_s = ti[beg:end], d[beg:end]
            n_pad = (cl[w] if half == 0 else ch[w]) * CHUNK
            pad = n_pad - len(t_s)
            t_p = np.concatenate([t_s, np.zeros(pad, np.int64)])
            r_p = np.concatenate([d_s - w * W, np.zeros(pad, np.int64)])
            m_p = np.concatenate(
                [np.zeros(len(t_s), np.float32), np.full(pad, MASKVAL, np.float32)]
            )
            return t_p, r_p, m_p

        idx_cols, mk_cols, m01_cols, m01t_cols = [], [], [], []
        for wp in range(NP):
            w0, w1 = 2 * wp, 2 * wp + 1
            segs = [seg(w0, 0), seg(w1, 0), seg(w0, 1), seg(w1, 1)]
            aa = np.concatenate([s[0] for s in segs])
            rr = np.concatenate([s[1] for s in segs])
            mm = np.concatenate([s[2] for s in segs])
            if len(aa):
                idx_cols.append(_pack_idx(aa))
            cpp = cl[w0] + cl[w1] + ch[w0] + ch[w1]
            mk_cols.append(mm.reshape(cpp, CHUNK).T.copy())
            rt = rr.reshape(cpp, CHUNK)                      # [k, e]
            m01 = (rt.T[:, :, None] == dcols).astype(ml_dtypes.bfloat16)
            m01_cols.append(m01.reshape(128, cpp * W))
            m01t = (dcols[:, None, None] == rt[None, :, :]).astype(ml_dtypes.bfloat16)
            m01t_pad = np.zeros((128, cpp, CHUNK), ml_dtypes.bfloat16)
            m01t_pad[:W] = m01t
            m01t_cols.append(m01t_pad.reshape(128, cpp * CHUNK))
        per_core.append(
            dict(
                IDX=np.concatenate(idx_cols, axis=1),
                MK=np.concatenate(mk_cols, axis=1).astype(np.float32),
                M01=np.concatenate(m01_cols, axis=1),
                M01T=np.concatenate(m01t_cols, axis=1),
            )
        )
    return cl, ch, per_core


def _build(cl, ch, icols, mkcols, mcols, mtcols):
    """Build the 8-core SPMD graph."""
    nc = bacc.Bacc(None, target_bir_lowering=False, debug=False, num_devices=NCORES)

    xs_in = nc.declare_dram_parameter("XS", [NSH, IN_F], f32, isOutput=False)
    wl_in = nc.declare_dram_parameter("WL", [IN_F, F], f32, isOutput=False)
    wr_in = nc.declare_dram_parameter("WR", [IN_F, F], f32, isOutput=False)
    attb_in = nc.declare_dram_parameter("ATTB", [128, F], f32, isOutput=False)
    gnp_in = nc.declare_dram_parameter("GNP", [C, 4], f32, isOutput=False)
    idx_in = nc.declare_dram_parameter("IDX", [128, icols], i16, isOutput=False)
    mk_in = nc.declare_dram_parameter("MK", [128, mkcols], f32, isOutput=False)
    m01_in = nc.declare_dram_parameter("M01", [128, mcols], bf16, isOutput=False)
    m01t_in = nc.declare_dram_parameter("M01T", [128, mtcols], bf16, isOutput=False)
    out_ext = nc.declare_dram_parameter("OUT", [NSH, C], f32, isOutput=True)

    xl_sha = nc.dram_tensor("xl_sha", [SPL, F], bf16)
    xl_shb = nc.dram_tensor("xl_shb", [NSH - SPL, F], bf16)
    xr_p = [nc.dram_tensor(f"xr_p{p}", [2 * W, F], bf16) for p in range(NP)]
    xl_fa = nc.dram_tensor("xl_fa", [NA, F], bf16, addr_space="Shared")
    xl_fb = nc.dram_tensor("xl_fb", [NB, F], bf16, addr_space="Shared")
    stats_l = nc.dram_tensor("stats_l", [C, 2], f32)
    stats_g = nc.dram_tensor("stats_g", [C, 2], f32, addr_space="Shared")

    # per-pair chunk counts and offsets
    cpp = [int(cl[2 * p] + cl[2 * p + 1] + ch[2 * p] + ch[2 * p + 1])
           for p in range(NP)]
    cpp_max = max(cpp)
    csum = np.concatenate([[0], np.cumsum(cpp)])
    ioff = csum * 8

    with tile.TileContext(nc) as tc:
        with (
            tc.tile_pool(name="const", bufs=1) as cp,
            tc.tile_pool(name="sb", bufs=2) as sb,
        ):
            ident = cp.tile([128, 128], f32)
            make_identity(nc, ident[:])
            identb = cp.tile([128, 128], bf16)
            nc.vector.tensor_copy(out=identb[:], in_=ident[:])
            wl_t = cp.tile([128, 2, F], bf16)
            wr_t = cp.tile([128, 2, F], bf16)
            wl_f = sb.tile([128, 2, F], f32, tag="wf")
            wr_f = sb.tile([128, 2, F], f32, tag="wf")
            nc.sync.dma_start(wl_f[:], wl_in.ap().rearrange("(s k) n -> k s n", k=128))
            nc.sync.dma_start(wr_f[:], wr_in.ap().rearrange("(s k) n -> k s n", k=128))
            nc.vector.tensor_copy(out=wl_t[:], in_=wl_f[:])
            nc.vector.tensor_copy(out=wr_t[:], in_=wr_f[:])
            # att replicated 4x along free dim for group ops
            attb_f = sb.tile([128, F], f32, tag="wf")
            nc.sync.dma_start(attb_f[:], attb_in[:, :])
            attg = cp.tile([128, GRP, F], bf16)
            for j in range(GRP):
                nc.vector.tensor_copy(out=attg[:, j, :], in_=attb_f[:])
            gnp_t = cp.tile([128, 4], f32)
            nc.sync.dma_start(gnp_t[:C, :], gnp_in[:, :])
            ones_n = cp.tile([128, 1], f32)
            nc.vector.memset(ones_n[:], 1.0)
            ones_r = cp.tile([128, W], f32)
            nc.vector.memset(ones_r[0:1, :], 1.0)
            biasb_t = cp.tile([128, C], f32)
            sb_t = cp.tile([128, C], f32)
            tb_t = cp.tile([128, C], f32)
            om_all = cp.tile([128, NW, C], f32)
            # cached X transposes for the xr pass
            xt_all = cp.tile([128, NTILE, 2, 128], bf16)
            # self-loop pad mask: rows 125..127 killed
            mk_self = cp.tile([128, 1], f32)
            nc.vector.memset(mk_self[:], MASKVAL)
            nc.vector.memset(mk_self[:W, :], 0.0)

            # biasB: [W, C] replicated GAT bias (gnp col 3)
            with tc.tile_pool(name="psi", bufs=1, space="PSUM") as psi:
                brow_ps = psi.tile([128, C], f32, space="PSUM", tag="brow")
                nc.tensor.transpose(brow_ps[0:1, :], gnp_t[:C, 3:4], ident[:C, :C])
                brow_t = cp.tile([128, C], f32)
                nc.vector.tensor_copy(out=brow_t[0:1, :], in_=brow_ps[0:1, :])
                biasb_ps = psi.tile([128, C], f32, space="PSUM", tag="bb")
                nc.tensor.matmul(
                    biasb_ps[:W, :], lhsT=ones_r[0:1, :], rhs=brow_t[0:1, :],
                    start=True, stop=True,
                )
                nc.vector.tensor_copy(out=biasb_t[:W, :], in_=biasb_ps[:W, :])

            # ---- P0: xr for the first pairs, then xl-A/AG-A, xl-B/AG-B,
            # then the rest of xr (self-healing vs queue hoisting) ----
            XGRP = 4          # tiles per batched X load
            NXG = (NTILE + XGRP - 1) // XGRP
            xb_all = {}

            with tc.tile_pool(name="ps0", bufs=2, space="PSUM") as ps0:
                def load_x(gi):
                    """Batched X load + bf16 cast for tiles [gi*XGRP, ...)."""
                    t0, t1 = gi * XGRP, min((gi + 1) * XGRP, NTILE)
                    rows = t1 * 128 - t0 * 128 if t1 * 128 <= NSH else NSH - t0 * 128
                    nt = t1 - t0
                    x_t = sb.tile([128, XGRP, IN_F], f32, tag="p0x", bufs=2)
                    if rows == nt * 128:
                        nc.sync.dma_start(
                            x_t[:, :nt, :],
                            xs_in.ap()[t0 * 128:t0 * 128 + rows, :].rearrange(
                                "(t p) f -> p t f", p=128
                            ),
                        )
                    else:
                        full = rows // 128
                        if full:
                            nc.sync.dma_start(
                                x_t[:, :full, :],
                                xs_in.ap()[t0 * 128:t0 * 128 + full * 128, :].rearrange(
                                    "(t p) f -> p t f", p=128
                                ),
                            )
                        rem = rows - full * 128
                        nc.sync.dma_start(
                            x_t[:rem, full, :],
                            xs_in[t0 * 128 + full * 128:t0 * 128 + rows, :],
                        )
                    xb_t = sb.tile([128, XGRP, IN_F], bf16, tag="p0xb", bufs=2)
                    nc.vector.tensor_copy(out=xb_t[:, :nt, :], in_=x_t[:, :nt, :])
                    return xb_t

                def transpose_tile(i, xb_t):
                    rows = min(128, NSH - i * 128)
                    for hh in range(2):
                        xt_ps = ps0.tile([128, 128], bf16, space="PSUM", tag="p0tp", bufs=4)
                        nc.tensor.transpose(
                            xt_ps[:, :rows],
                            xb_t[:rows, i % XGRP, hh * 128:(hh + 1) * 128],
                            identb[:rows, :rows],
                        )
                        nc.vector.tensor_copy(
                            out=xt_all[:, i, hh, :rows], in_=xt_ps[:, :rows]
                        )

                def xw_tile(i, w_t, store):
                    """matmul against cached transpose + store callback"""
                    rows = min(128, NSH - i * 128)
                    mm_ps = ps0.tile([128, F], f32, space="PSUM", tag="p0mm", bufs=4)
                    for hh in range(2):
                        nc.tensor.matmul(
                            mm_ps[:rows, :], lhsT=xt_all[:, i, hh, :rows],
                            rhs=w_t[:, hh, :], start=(hh == 0), stop=(hh == 1),
                        )
                    o_t = sb.tile([128, F], bf16, tag="p0o", bufs=4)
                    nc.vector.tensor_copy(out=o_t[:rows, :], in_=mm_ps[:rows, :])
                    store(i, rows, o_t)

                def store_xr(i, rows, o_t):
                    r0 = i * 128
                    while r0 < i * 128 + rows:
                        p = r0 // (2 * W)
                        r1 = min((p + 1) * 2 * W, i * 128 + rows)
                        nc.sync.dma_start(
                            xr_p[p][r0 - p * 2 * W:r1 - p * 2 * W, :],
                            o_t[r0 - i * 128:r1 - i * 128, :],
                        )
                        r0 = r1

                def store_xl(i, rows, o_t):
                    if i * 128 < SPL:
                        xl_dram, xl_row = xl_sha, i * 128
                    else:
                        xl_dram, xl_row = xl_shb, i * 128 - SPL
                    nc.sync.dma_start(xl_dram[xl_row:xl_row + rows, :], o_t[:rows, :])

                # pass 1: xl table A (with transposes) -> AG-A asap
                for gi in range((SPLT + XGRP - 1) // XGRP):
                    t0, t1 = gi * XGRP, min((gi + 1) * XGRP, SPLT)
                    xb = load_x(gi)
                    xb_all[gi] = xb
                    for i in range(t0, t1):
                        transpose_tile(i, xb)
                        xw_tile(i, wl_t, store_xl)

                nc.gpsimd.collective_compute(
                    "AllGather", mybir.AluOpType.bypass,
                    replica_groups=[list(range(NCORES))],
                    ins=[xl_sha.ap().opt()], outs=[xl_fa.ap().opt()],
                )
                # pass 2: xl table B (with transposes) -> AG-B
                for gi in range(NXG):
                    t0, t1 = gi * XGRP, min((gi + 1) * XGRP, NTILE)
                    if t1 <= SPLT:
                        continue
                    xb = xb_all.get(gi) or load_x(gi)
                    xb_all[gi] = xb
                    for i in range(max(t0, SPLT), t1):
                        transpose_tile(i, xb)
                        xw_tile(i, wl_t, store_xl)
                nc.gpsimd.collective_compute(
                    "AllGather", mybir.AluOpType.bypass,
                    replica_groups=[list(range(NCORES))],
                    ins=[xl_shb.ap().opt()], outs=[xl_fb.ap().opt()],
                )
                # pass 3: xr (transposes all cached; writes on the clean sync
                # queue, P2 loads live on the scalar queue so nothing blocks)
                for i in range(NTILE):
                    xw_tile(i, wr_t, store_xr)

            # ---- P2: paired-window edge processing ----
            with (
                tc.tile_pool(name="acc", bufs=1, space="PSUM") as accp,
                tc.tile_pool(name="ps2", bufs=2, space="PSUM") as ps2,
            ):
                sum_ps = accp.tile([128, 1], f32, space="PSUM", tag="accsum")
                ssq_ps = accp.tile([128, 1], f32, space="PSUM", tag="accssq")
                def issue_gather(wp):
                    """ix load + both gathers for pair wp (lookahead side)."""
                    w0, w1 = 2 * wp, 2 * wp + 1
                    nLp = int(cl[w0] + cl[w1])
                    nHp = int(ch[w0] + ch[w1])
                    nTp = cpp[wp]
                    ix_t = sb.tile([128, cpp_max * 8], i16, tag="ix", bufs=LA + 1)
                    nc.scalar.dma_start(
                        ix_t[:, :nTp * 8], idx_in[:, int(ioff[wp]):int(ioff[wp + 1])]
                    )
                    a_t = sb.tile([128, cpp_max, F], bf16, tag="a", bufs=LA + 1)
                    if nLp:
                        nc.gpsimd.dma_gather(
                            a_t[:, :nLp, :], xl_fa[:, :], ix_t[:, :nLp * 8],
                            nLp * CHUNK, nLp * CHUNK, F,
                            single_packet=(nLp * CHUNK <= 1024),
                        )
                    if nHp:
                        nc.gpsimd.dma_gather(
                            a_t[:, nLp:nTp, :], xl_fb[:, :],
                            ix_t[:, nLp * 8:nTp * 8], nHp * CHUNK, nHp * CHUNK, F,
                            single_packet=(nHp * CHUNK <= 1024),
                        )
                    return a_t

                def issue_masks(wp):
                    """mask/xr/self loads for pair wp (lookahead 1)."""
                    w0, w1 = 2 * wp, 2 * wp + 1
                    nTp = cpp[wp]
                    mk_t = sb.tile([128, cpp_max], f32, tag="mk", bufs=2)
                    nc.scalar.dma_start(
                        mk_t[:, :nTp], mk_in[:, int(csum[wp]):int(csum[wp + 1])]
                    )
                    m01_t = sb.tile([128, cpp_max, W], bf16, tag="m01", bufs=2)
                    nc.scalar.dma_start(
                        m01_t[:, :nTp, :],
                        m01_in[:, int(csum[wp]) * W:int(csum[wp + 1]) * W],
                    )
                    m01t_t = sb.tile([128, cpp_max, CHUNK], bf16, tag="m01t", bufs=2)
                    nc.scalar.dma_start(
                        m01t_t[:, :nTp, :],
                        m01t_in[:, int(csum[wp]) * CHUNK:int(csum[wp + 1]) * CHUNK],
                    )
                    xrq_t = sb.tile([128, 2, F], bf16, tag="xrq", bufs=2)
                    nc.scalar.dma_start(
                        xrq_t[:W, :, :],
                        xr_p[wp].ap().rearrange("(t p) f -> p t f", p=W),
                    )
                    a_self = sb.tile([128, 2, F], bf16, tag="aself", bufs=2)
                    if wp < 2:
                        # init both ring buffers once; rows W:128 must stay 0
                        nc.vector.memset(a_self[:], 0.0)
                    for wi, w in ((0, w0), (1, w1)):
                        r0 = w * W
                        if r0 + W <= SPL:
                            nc.scalar.dma_start(
                                a_self[:W, wi, :], xl_sha[r0:r0 + W, :]
                            )
                        elif r0 >= SPL:
                            nc.scalar.dma_start(
                                a_self[:W, wi, :], xl_shb[r0 - SPL:r0 - SPL + W, :]
                            )
                        else:
                            na = SPL - r0
                            nc.scalar.dma_start(a_self[:na, wi, :], xl_sha[r0:SPL, :])
                            nc.scalar.dma_start(
                                a_self[na:W, wi, :], xl_shb[0:W - na, :]
                            )
                    return mk_t, m01_t, m01t_t, xrq_t, a_self

                LA = 3
                gh = {}
                mh = {}
                for q in range(min(LA, NP)):
                    gh[q] = issue_gather(q)
                mh[0] = issue_masks(0)
                for wp in range(NP):
                    if wp + LA < NP:
                        gh[wp + LA] = issue_gather(wp + LA)
                    if wp + 1 < NP:
                        mh[wp + 1] = issue_masks(wp + 1)
                    a_t = gh.pop(wp)
                    mk_t, m01_t, m01t_t, xrq_t, a_self = mh.pop(wp)
                    w0, w1 = 2 * wp, 2 * wp + 1
                    nL0, nL1 = int(cl[w0]), int(cl[w1])
                    nH0, nH1 = int(ch[w0]), int(ch[w1])
                    nLp = nL0 + nL1
                    nHp = nH0 + nH1
                    nTp = cpp[wp]
                    out_pair = ps2.tile(
                        [W, 2, 512], f32, space="PSUM", tag="outp", bufs=1,
                    )
                    done = {w0: 0, w1: 0}
                    for wi, w in ((0, w0), (1, w1)):
                        # chunk ranges of window w within the pair layout
                        if wi == 0:
                            ranges = [(0, nL0), (nLp, nLp + nH0)]
                        else:
                            ranges = [(nL0, nLp), (nLp + nH0, nTp)]
                        for rb, re in ranges:
                            for k0 in range(rb, re, GRP):
                                g = min(GRP, re - k0)
                                l4 = sb.tile([128, GRP, F], bf16, tag="l4")
                                for j in range(g):
                                    k = k0 + j
                                    b_ps = ps2.tile(
                                        [128, F], f32, space="PSUM", tag="bps",
                                        bufs=3,
                                    )
                                    nc.tensor.matmul(
                                        b_ps[:], lhsT=m01t_t[:W, k, :],
                                        rhs=xrq_t[:W, wi, :],
                                        start=True, stop=False,
                                    )
                                    nc.tensor.matmul(
                                        b_ps[:], lhsT=identb[:],
                                        rhs=a_t[:, k, :],
                                        start=False, stop=True,
                                    )
                                    nc.scalar.activation(
                                        l4[:, j, :], b_ps[:],
                                        mybir.ActivationFunctionType.Prelu, alpha=NEG,
                                    )
                                ap4 = sb.tile([128, GRP, FQ], bf16, tag="ap4")
                                t4 = sb.tile([128, GRP, F], bf16, tag="t4")
                                nc.vector.tensor_tensor(
                                    out=t4[:, :g, :], in0=l4[:, :g, :],
                                    in1=attg[:, :g, :], op=mybir.AluOpType.mult,
                                )
                                lg4 = sb.tile([128, GRP, H], f32, tag="lg4")
                                nc.vector.reduce_sum(
                                    out=lg4[:, :g, :],
                                    in_=t4[:, :g, :].rearrange("p k (h c) -> p k h c", h=H),
                                    axis=mybir.AxisListType.X,
                                )
                                lgm4 = sb.tile([128, GRP, H], f32, tag="lgm4")
                                nc.vector.tensor_tensor(
                                    out=lgm4[:, :g, :], in0=lg4[:, :g, :],
                                    in1=mk_t[:, k0:k0 + g].rearrange(
                                        "p (g o) -> p g o", o=1
                                    ).to_broadcast([128, g, H]),
                                    op=mybir.AluOpType.add,
                                )
                                nc.scalar.activation(
                                    ap4[:, :g, F:FQ], lgm4[:, :g, :],
                                    mybir.ActivationFunctionType.Exp,
                                )
                                nc.vector.tensor_tensor(
                                    out=ap4[:, :g, 0:F].rearrange("p k (h c) -> p k h c", h=H),
                                    in0=a_t[:, k0:k0 + g, :].rearrange("p k (h c) -> p k h c", h=H),
                                    in1=ap4[:, :g, F:FQ].rearrange("p k (h o) -> p k h o", o=1).to_broadcast([128, g, H, C]),
                                    op=mybir.AluOpType.mult,
                                )
                                for j in range(g):
                                    done[w] += 1
                                    nc.tensor.matmul(
                                        out_pair[:, wi, 0:FQ],
                                        lhsT=m01_t[:, k0 + j, :],
                                        rhs=ap4[:, j, :],
                                        start=(done[w] == 1), stop=False,
                                    )
                    # --- per-window self-loop chunk (identity one-hots) ---
                    for wi, w in ((0, w0), (1, w1)):
                        bs_ps = ps2.tile([128, F], f32, space="PSUM", tag="bsps",
                                         bufs=1)
                        nc.tensor.matmul(
                            bs_ps[:], lhsT=identb[:W, :],
                            rhs=xrq_t[:W, wi, :], start=True, stop=False,
                        )
                        nc.tensor.matmul(
                            bs_ps[:], lhsT=identb[:], rhs=a_self[:, wi, :],
                            start=False, stop=True,
                        )
                        l_s = sb.tile([128, F], bf16, tag="ls", bufs=2)
                        nc.scalar.activation(
                            l_s[:], bs_ps[:],
                            mybir.ActivationFunctionType.Prelu, alpha=NEG,
                        )
                        t_s = sb.tile([128, F], bf16, tag="ts", bufs=2)
                        nc.vector.tensor_tensor(
                            out=t_s[:], in0=l_s[:], in1=attg[:, 0, :],
                            op=mybir.AluOpType.mult,
                        )
                        lg_s = sb.tile([128, H], f32, tag="lgs", bufs=2)
                        nc.vector.reduce_sum(
                            out=lg_s[:],
                            in_=t_s[:].rearrange("p (h c) -> p h c", h=H),
                            axis=mybir.AxisListType.X,
                        )
                        ap_s = sb.tile([128, FQ], bf16, tag="aps", bufs=2)
                        nc.scalar.activation(
                            ap_s[:, F:FQ], lg_s[:],
                            mybir.ActivationFunctionType.Exp,
                            bias=mk_self[:], scale=1.0,
                        )
                        nc.vector.tensor_tensor(
                            out=ap_s[:, 0:F].rearrange("p (h c) -> p h c", h=H),
                            in0=a_self[:, wi, :].rearrange("p (h c) -> p h c", h=H),
                            in1=ap_s[:, F:FQ].rearrange("p (h o) -> p h o", o=1).to_broadcast([128, H, C]),
                            op=mybir.AluOpType.mult,
                        )
                        nc.tensor.matmul(
                            out_pair[:, wi, 0:FQ], lhsT=identb[:, :W], rhs=ap_s[:],
                            start=False, stop=True,
                        )
                    # --- pair-batched epilogue ---
                    rd_t = sb.tile([128, 2, H], f32, tag="rd")
                    nc.vector.reciprocal(rd_t[:W, :, :], out_pair[:, :, F:FQ])
                    oh_t = sb.tile([128, 2, H, C], f32, tag="oh")
                    for wi in (0, 1):
                        for h in range(H):
                            nc.vector.tensor_scalar(
                                out=oh_t[:W, wi, h, :],
                                in0=out_pair[:, wi, h * C:(h + 1) * C],
                                scalar1=rd_t[:W, wi, h:h + 1], scalar2=0.25,
                                op0=mybir.AluOpType.mult, op1=mybir.AluOpType.mult,
                            )
                    o01 = sb.tile([128, 2, C], f32, tag="o01")
                    nc.vector.tensor_tensor(
                        out=o01[:W, :, :], in0=oh_t[:W, :, 0, :], in1=oh_t[:W, :, 1, :],
                        op=mybir.AluOpType.add,
                    )
                    o23 = sb.tile([128, 2, C], f32, tag="o23")
                    nc.vector.tensor_tensor(
                        out=o23[:W, :, :], in0=oh_t[:W, :, 2, :], in1=oh_t[:W, :, 3, :],
                        op=mybir.AluOpType.add,
                    )
                    o0123 = sb.tile([128, 2, C], f32, tag="o0123")
                    nc.vector.tensor_tensor(
                        out=o0123[:W, :, :], in0=o01[:W, :, :], in1=o23[:W, :, :],
                        op=mybir.AluOpType.add,
                    )
                    nc.vector.tensor_tensor(
                        out=om_all[:W, w0:w0 + 2, :], in0=o0123[:W, :, :],
                        in1=biasb_t[:W, :].rearrange("p (o c) -> p o c", o=1).to_broadcast([W, 2, C]),
                        op=mybir.AluOpType.add,
                    )
                    sq_t = sb.tile([128, 2, C], f32, tag="sq")
                    nc.scalar.square(sq_t[:W, :, :], om_all[:W, w0:w0 + 2, :])
                    for wi, w in ((0, w0), (1, w1)):
                        nc.tensor.matmul(
                            sum_ps[:C, :], lhsT=om_all[:W, w, :], rhs=ones_n[:W, :],
                            start=(w == 0), stop=(w == NW - 1),
                        )
                        nc.tensor.matmul(
                            ssq_ps[:C, :], lhsT=sq_t[:W, wi, :], rhs=ones_n[:W, :],
                            start=(w == 0), stop=(w == NW - 1),
                        )

            # ---- P3: GraphNorm statistics ----
            st_t = sb.tile([128, 2], f32, tag="st")
            nc.vector.tensor_copy(out=st_t[:C, 0:1], in_=sum_ps[:C, :])
            nc.vector.tensor_copy(out=st_t[:C, 1:2], in_=ssq_ps[:C, :])
            nc.sync.dma_start(stats_l[:, :], st_t[:C, :])
            nc.gpsimd.collective_compute(
                "AllReduce", mybir.AluOpType.add,
                replica_groups=[list(range(NCORES))],
                ins=[stats_l.ap().opt()], outs=[stats_g.ap().opt()],
            )
            sg_t = sb.tile([128, 2], f32, tag="sg")
            nc.sync.dma_start(sg_t[:C, :], stats_g[:, :])
            mu_t = sb.tile([128, 1], f32, tag="mu")
            nc.vector.tensor_scalar_mul(mu_t[:C, :], sg_t[:C, 0:1], 1.0 / N)
            msq_t = sb.tile([128, 1], f32, tag="msq")
            nc.vector.tensor_scalar_mul(msq_t[:C, :], sg_t[:C, 1:2], 1.0 / N)
            amu_t = sb.tile([128, 1], f32, tag="amu")
            nc.vector.tensor_tensor(
                out=amu_t[:C, :], in0=gnp_t[:C, 2:3], in1=mu_t[:C, :],
                op=mybir.AluOpType.mult,
            )
            am2_t = sb.tile([128, 1], f32, tag="am2")
            nc.vector.scalar_tensor_tensor(
                out=am2_t[:C, :], in0=mu_t[:C, :], scalar=-2.0, in1=amu_t[:C, :],
                op0=mybir.AluOpType.mult, op1=mybir.AluOpType.add,
            )
            var_t = sb.tile([128, 1], f32, tag="var")
            nc.vector.tensor_tensor(
                out=var_t[:C, :], in0=amu_t[:C, :], in1=am2_t[:C, :],
                op=mybir.AluOpType.mult,
            )
            nc.vector.tensor_tensor(
                out=var_t[:C, :], in0=var_t[:C, :], in1=msq_t[:C, :],
                op=mybir.AluOpType.add,
            )
            nc.vector.tensor_scalar_add(var_t[:C, :], var_t[:C, :], EPS)
            sd_t = sb.tile([128, 1], f32, tag="sd")
            nc.scalar.sqrt(sd_t[:C, :], var_t[:C, :])
            inv_t = sb.tile([128, 1], f32, tag="inv")
            nc.vector.reciprocal(inv_t[:C, :], sd_t[:C, :])
            st2_t = sb.tile([128, 2], f32, tag="st2")
            nc.vector.tensor_tensor(
                out=st2_t[:C, 0:1], in0=gnp_t[:C, 0:1], in1=inv_t[:C, :],
                op=mybir.AluOpType.mult,
            )
            u_t = sb.tile([128, 1], f32, tag="u")
            nc.vector.tensor_tensor(
                out=u_t[:C, :], in0=st2_t[:C, 0:1], in1=amu_t[:C, :],
                op=mybir.AluOpType.mult,
            )
            nc.vector.tensor_tensor(
                out=st2_t[:C, 1:2], in0=gnp_t[:C, 1:2], in1=u_t[:C, :],
                op=mybir.AluOpType.subtract,
            )
            with tc.tile_pool(name="ps3", bufs=1, space="PSUM") as ps3:
                srow_ps = ps3.tile([128, C], f32, space="PSUM", tag="srow")
                nc.tensor.transpose(srow_ps[0:1, :], st2_t[:C, 0:1], ident[:C, :C])
                trow_ps = ps3.tile([128, C], f32, space="PSUM", tag="trow")
                nc.tensor.transpose(trow_ps[0:1, :], st2_t[:C, 1:2], ident[:C, :C])
                srow_t = sb.tile([128, C], f32, tag="srow")
                nc.vector.tensor_copy(out=srow_t[0:1, :], in_=srow_ps[0:1, :])
                trow_t = sb.tile([128, C], f32, tag="trow")
                nc.vector.tensor_copy(out=trow_t[0:1, :], in_=trow_ps[0:1, :])
                sb_ps = ps3.tile([128, C], f32, space="PSUM", tag="sbp")
                nc.tensor.matmul(
                    sb_ps[:W, :], lhsT=ones_r[0:1, :], rhs=srow_t[0:1, :],
                    start=True, stop=True,
                )
                nc.vector.tensor_copy(out=sb_t[:W, :], in_=sb_ps[:W, :])
                tb_ps = ps3.tile([128, C], f32, space="PSUM", tag="tbp")
                nc.tensor.matmul(
                    tb_ps[:W, :], lhsT=ones_r[0:1, :], rhs=trow_t[0:1, :],
                    start=True, stop=True,
                )
                nc.vector.tensor_copy(out=tb_t[:W, :], in_=tb_ps[:W, :])

            # ---- P4: apply GraphNorm affine in window chunks (overlap DMA) ----
            WCH = 10
            for c0 in range(0, NW, WCH):
                y_t = sb.tile([128, WCH, C], f32, tag="y", bufs=2)
                nc.vector.tensor_tensor(
                    out=y_t[:W, :, :], in0=om_all[:W, c0:c0 + WCH, :],
                    in1=sb_t[:W, :].rearrange("p (o c) -> p o c", o=1).to_broadcast([W, WCH, C]),
                    op=mybir.AluOpType.mult,
                )
                nc.vector.tensor_tensor(
                    out=y_t[:W, :, :], in0=y_t[:W, :, :],
                    in1=tb_t[:W, :].rearrange("p (o c) -> p o c", o=1).to_broadcast([W, WCH, C]),
                    op=mybir.AluOpType.add,
                )
                nc.sync.dma_start(
                    out_ext.ap()[c0 * W:(c0 + WCH) * W, :].rearrange(
                        "(w p) c -> p w c", p=W
                    ),
                    y_t[:W, :, :],
                )
    nc.compile()
    return nc


def kernel(X, E, Wl, Wr, att, bias, gn_weight, gn_bias, gn_mean_scale, **kw):
    global LAST_RESULTS
    X = np.asarray(X, np.float32)
    E = np.asarray(E)
    Wl = np.asarray(Wl, np.float32)
    Wr = np.asarray(Wr, np.float32)
    att = np.asarray(att, np.float32)
    bias = np.asarray(bias, np.float32)
    gn_weight = np.asarray(gn_weight, np.float32)
    gn_bias = np.asarray(gn_bias, np.float32)
    gn_mean_scale = np.asarray(gn_mean_scale, np.float32)

    # self-loops handled separately on-device; only random edges here
    src = np.asarray(E[0], np.int64)
    dst = np.asarray(E[1], np.int64)
    cl, ch, per_core = _prep_edges(src, dst)

    attb = np.tile(att.reshape(1, F), (128, 1)).astype(np.float32)
    gnp = np.stack([gn_weight, gn_bias, gn_mean_scale, bias], axis=1).astype(np.float32)

    p0 = per_core[0]
    nc = _build(cl, ch, p0["IDX"].shape[1], p0["MK"].shape[1],
                p0["M01"].shape[1], p0["M01T"].shape[1])

    in_maps = []
    for c in range(NCORES):
        in_maps.append(
            dict(
                XS=np.ascontiguousarray(X[c * NSH:(c + 1) * NSH]),
                WL=Wl, WR=Wr, ATTB=attb, GNP=gnp,
                IDX=per_core[c]["IDX"], MK=per_core[c]["MK"],
                M01=per_core[c]["M01"], M01T=per_core[c]["M01T"],
            )
        )
    trace = bool(kw.get("trace"))
    res = run_bass_kernel_spmd(
        nc, in_maps, core_ids=list(range(NCORES)), trace=trace
    )
    LAST_RESULTS = res
    return np.concatenate([res.results[c]["OUT"] for c in range(NCORES)], axis=0)


# revision 21
# speedup vs baseline: 1.5660x; 1.5660x over previous
"""GATv2Conv (heads=4, concat=False, self-loops) + GraphNorm on 8 TRN2 NeuronCores.

v5 design notes (on top of v4):
- The Pool-engine SWDGE gather is the span bottleneck (~6.7ns/row of Q7
  descriptor generation, independent of DMA-queue load), so v5 attacks
  gather ROWS and fixed overheads:
  * Self-loop edges (125/window, contiguous local rows) leave the gather:
    their xl rows arrive via HWDGE dma_start and their one-hots are the
    static identity, so 6250 rows/core disappear from the gather.
  * Windows are processed in PAIRS with one gather per table per pair,
    halving the ~1us fixed cost per dma_gather. (Pad slots keep index 0:
    negative-skip needs trailing-only pads + per-core num_idxs_reg.)
- Per-GRP EXP is batched into a single ACT op (mask added on DVE).
- Everything else (edge-major chunks, one-hot matmul pick/scatter, PSUM
  accumulation, GraphNorm via AllReduce) is unchanged from v4.
"""
import os
import sys

sys.path.insert(0, "/opt/trn_rl_repo")

import ml_dtypes
import numpy as np
from concourse import bacc, mybir, tile
from concourse.bass_utils import run_bass_kernel_spmd
from concourse.masks import make_identity

N = 50000
NCORES = 8
NSH = N // NCORES          # 6250 dst nodes per core
SPL = 3200                 # sub-shard split: rows [0:SPL) -> table A
NA = NCORES * SPL          # 25600 rows in table A
NB = NCORES * (NSH - SPL)  # 24400 rows in table B
IN_F = 256
H = 4
C = 64
F = H * C                  # 256
FQ = F + H                 # 260: scatter rhs = [a*p || p]
W = 125                    # dst window size
NW = NSH // W              # 50 windows per core
NP = NW // 2               # 25 window pairs
NEG = 0.2
EPS = 1e-5
MASKVAL = -100.0           # logit bias for padding edges -> exp == 0
CHUNK = 128
GRP = 4                    # chunks fused per DVE op group

f32 = mybir.dt.float32
bf16 = mybir.dt.bfloat16
i16 = mybir.dt.int16

LAST_RESULTS = None


def _pack_idx(idx: np.ndarray) -> np.ndarray:
    """[n] int -> [128, n//16] int16 gather-index layout (16-partition wrap,
    replicated for the 8 Q7 cores)."""
    n = idx.shape[0]
    pk = np.zeros((16, n // 16), np.int16)
    pk[np.arange(n) % 16, np.arange(n) // 16] = idx.astype(np.int16)
    return np.tile(pk, (8, 1))


def _prep_edges(src: np.ndarray, dst: np.ndarray):
    """Partition/sort/pad the RANDOM edges (self-loops excluded by caller).
    Pair layout per window pair wp=(2w, 2w+1):
      chunks [lo(2w) | lo(2w+1) | hi(2w) | hi(2w+1)]
    so one gather per table covers both windows. Pad slots get index -1
    (skipped by dma_gather) and mask -100.
    Returns (cl, ch, per_core) with IDX/MK/M01/M01T in pair-chunk order."""
    src = src.astype(np.int64)
    dst = dst.astype(np.int64)
    core = dst // NSH
    scid = src // NSH                  # source core
    soff = src % NSH                   # offset within source shard
    is_b = soff >= SPL
    tidx = np.where(is_b, scid * (NSH - SPL) + soff - SPL, scid * SPL + soff)
    per_core_raw = []
    nlo = np.zeros((NCORES, NW), np.int64)
    nhi = np.zeros((NCORES, NW), np.int64)
    for c in range(NCORES):
        m = core == c
        ti = tidx[m]
        hb = is_b[m].astype(np.int64)
        d = dst[m] - c * NSH
        win = d // W
        order = np.lexsort((hb, win))
        ti, d, hb = ti[order], d[order], hb[order]
        key = (d // W) * 2 + hb
        cnt = np.bincount(key, minlength=NW * 2).reshape(NW, 2)
        nlo[c] = cnt[:, 0]
        nhi[c] = cnt[:, 1]
        per_core_raw.append((ti, d, np.cumsum(cnt.reshape(-1))))
    cl = np.ceil(nlo.max(axis=0) / CHUNK).astype(int)
    ch = np.ceil(nhi.max(axis=0) / CHUNK).astype(int)

    dcols = np.arange(W, dtype=np.int64)
    per_core = []
    for c in range(NCORES):
        ti, d, cum = per_core_raw[c]

        def seg(w, half):
            """(tidx, local-d) arrays for (window, half) padded to chunks."""
            k = w * 2 + half
            beg = cum[k - 1] if k > 0 else 0
            end = cum[k]
            t_s, d_s = ti[beg:end], d[beg:end]
            n_pad = (cl[w] if half == 0 else ch[w]) * CHUNK
            pad = n_pad - len(t_s)
            t_p = np.concatenate([t_s, np.zeros(pad, np.int64)])
            r_p = np.concatenate([d_s - w * W, np.zeros(pad, np.int64)])
            m_p = np.concatenate(
                [np.zeros(len(t_s), np.float32), np.full(pad, MASKVAL, np.float32)]
            )
            return t_p, r_p, m_p

        idx_cols, mk_cols, m01_cols, m01t_cols = [], [], [], []
        for wp in range(NP):
            w0, w1 = 2 * wp, 2 * wp + 1
            segs = [seg(w0, 0), seg(w1, 0), seg(w0, 1), seg(w1, 1)]
            aa = np.concatenate([s[0] for s in segs])
            rr = np.concatenate([s[1] for s in segs])
            mm = np.concatenate([s[2] for s in segs])
            if len(aa):
                idx_cols.append(_pack_idx(aa))
            cpp = cl[w0] + cl[w1] + ch[w0] + ch[w1]
            mk_cols.append(mm.reshape(cpp, CHUNK).T.copy())
            rt = rr.reshape(cpp, CHUNK)                      # [k, e]
            m01 = (rt.T[:, :, None] == dcols).astype(ml_dtypes.bfloat16)
            m01_cols.append(m01.reshape(128, cpp * W))
            m01t = (dcols[:, None, None] == rt[None, :, :]).astype(ml_dtypes.bfloat16)
            m01t_pad = np.zeros((128, cpp, CHUNK), ml_dtypes.bfloat16)
            m01t_pad[:W] = m01t
            m01t_cols.append(m01t_pad.reshape(128, cpp * CHUNK))
        per_core.append(
            dict(
                IDX=np.concatenate(idx_cols, axis=1),
                MK=np.concatenate(mk_cols, axis=1).astype(np.float32),
                M01=np.concatenate(m01_cols, axis=1),
                M01T=np.concatenate(m01t_cols, axis=1),
            )
        )
    return cl, ch, per_core


def _build(cl, ch, icols, mkcols, mcols, mtcols):
    """Build the 8-core SPMD graph."""
    nc = bacc.Bacc(None, target_bir_lowering=False, debug=False, num_devices=NCORES)

    xs_in = nc.declare_dram_parameter("XS", [NSH, IN_F], f32, isOutput=False)
    wl_in = nc.declare_dram_parameter("WL", [IN_F, F], f32, isOutput=False)
    wr_in = nc.declare_dram_parameter("WR", [IN_F, F], f32, isOutput=False)
    attb_in = nc.declare_dram_parameter("ATTB", [128, F], f32, isOutput=False)
    gnp_in = nc.declare_dram_parameter("GNP", [C, 4], f32, isOutput=False)
    idx_in = nc.declare_dram_parameter("IDX", [128, icols], i16, isOutput=False)
    mk_in = nc.declare_dram_parameter("MK", [128, mkcols], f32, isOutput=False)
    m01_in = nc.declare_dram_parameter("M01", [128, mcols], bf16, isOutput=False)
    m01t_in = nc.declare_dram_parameter("M01T", [128, mtcols], bf16, isOutput=False)
    out_ext = nc.declare_dram_parameter("OUT", [NSH, C], f32, isOutput=True)

    xl_sha = nc.dram_tensor("xl_sha", [SPL, F], bf16)
    xl_shb = nc.dram_tensor("xl_shb", [NSH - SPL, F], bf16)
    xr_d = nc.dram_tensor("xr_d", [NSH, F], bf16)
    xl_fa = nc.dram_tensor("xl_fa", [NA, F], bf16, addr_space="Shared")
    xl_fb = nc.dram_tensor("xl_fb", [NB, F], bf16, addr_space="Shared")
    stats_l = nc.dram_tensor("stats_l", [C, 2], f32)
    stats_g = nc.dram_tensor("stats_g", [C, 2], f32, addr_space="Shared")

    # per-pair chunk counts and offsets
    cpp = [int(cl[2 * p] + cl[2 * p + 1] + ch[2 * p] + ch[2 * p + 1])
           for p in range(NP)]
    cpp_max = max(cpp)
    csum = np.concatenate([[0], np.cumsum(cpp)])
    ioff = csum * 8
    dbg_om = bool(int(os.environ.get("DBG_OM", "0")))

    with tile.TileContext(nc) as tc:
        with (
            tc.tile_pool(name="const", bufs=1) as cp,
            tc.tile_pool(name="sb", bufs=2) as sb,
            tc.tile_pool(name="acc", bufs=1, space="PSUM") as accp,
        ):
            ident = cp.tile([128, 128], f32)
            make_identity(nc, ident[:])
            identb = cp.tile([128, 128], bf16)
            nc.vector.tensor_copy(out=identb[:], in_=ident[:])
            wl_t = cp.tile([128, 2, F], bf16)
            wr_t = cp.tile([128, 2, F], bf16)
            wl_f = cp.tile([128, 2, F], f32)
            wr_f = cp.tile([128, 2, F], f32)
            nc.sync.dma_start(wl_f[:], wl_in.ap().rearrange("(s k) n -> k s n", k=128))
            nc.sync.dma_start(wr_f[:], wr_in.ap().rearrange("(s k) n -> k s n", k=128))
            nc.vector.tensor_copy(out=wl_t[:], in_=wl_f[:])
            nc.vector.tensor_copy(out=wr_t[:], in_=wr_f[:])
            # att replicated 4x along free dim for group ops
            attb_f = cp.tile([128, F], f32)
            nc.sync.dma_start(attb_f[:], attb_in[:, :])
            attg = cp.tile([128, GRP, F], bf16)
            for j in range(GRP):
                nc.vector.tensor_copy(out=attg[:, j, :], in_=attb_f[:])
            gnp_t = cp.tile([128, 4], f32)
            nc.sync.dma_start(gnp_t[:C, :], gnp_in[:, :])
            ones_n = cp.tile([128, 1], f32)
            nc.vector.memset(ones_n[:], 1.0)
            ones_r = cp.tile([128, W], f32)
            nc.vector.memset(ones_r[0:1, :], 1.0)
            biasb_t = cp.tile([128, C], f32)
            sb_t = cp.tile([128, C], f32)
            tb_t = cp.tile([128, C], f32)
            om_all = cp.tile([128, NW, C], f32)
            # self-loop pad mask: rows 125..127 killed
            mk_self = cp.tile([128, 1], f32)
            nc.vector.memset(mk_self[:], MASKVAL)
            nc.vector.memset(mk_self[:W, :], 0.0)

            # biasB: [W, C] replicated GAT bias (gnp col 3)
            with tc.tile_pool(name="psi", bufs=1, space="PSUM") as psi:
                brow_ps = psi.tile([128, C], f32, space="PSUM", tag="brow")
                nc.tensor.transpose(brow_ps[0:1, :], gnp_t[:C, 3:4], ident[:C, :C])
                brow_t = cp.tile([128, C], f32)
                nc.vector.tensor_copy(out=brow_t[0:1, :], in_=brow_ps[0:1, :])
                biasb_ps = psi.tile([128, C], f32, space="PSUM", tag="bb")
                nc.tensor.matmul(
                    biasb_ps[:W, :], lhsT=ones_r[0:1, :], rhs=brow_t[0:1, :],
                    start=True, stop=True,
                )
                nc.vector.tensor_copy(out=biasb_t[:W, :], in_=biasb_ps[:W, :])

            # ---- P0: xl/xr transforms (bf16) ----
            ntile = (NSH + 127) // 128
            with tc.tile_pool(name="ps0", bufs=2, space="PSUM") as ps0:
                for i in range(ntile):
                    rows = min(128, NSH - i * 128)
                    x_t = sb.tile([128, IN_F], f32, tag="p0x")
                    nc.sync.dma_start(x_t[:rows, :], xs_in[i * 128:i * 128 + rows, :])
                    xb_t = sb.tile([128, IN_F], bf16, tag="p0xb")
                    nc.vector.tensor_copy(out=xb_t[:rows, :], in_=x_t[:rows, :])
                    xt_sb = sb.tile([128, 2, 128], bf16, tag="p0xt")
                    for hh in range(2):
                        xt_ps = ps0.tile([128, 128], bf16, space="PSUM", tag="p0tp")
                        nc.tensor.transpose(
                            xt_ps[:, :rows], xb_t[:rows, hh * 128:(hh + 1) * 128],
                            identb[:rows, :rows],
                        )
                        nc.vector.tensor_copy(out=xt_sb[:, hh, :rows], in_=xt_ps[:, :rows])
                    if i * 128 < SPL:
                        xl_dram, xl_row = xl_sha, i * 128
                    else:
                        xl_dram, xl_row = xl_shb, i * 128 - SPL
                    for w_t, dram, row0 in ((wl_t, xl_dram, xl_row), (wr_t, xr_d, i * 128)):
                        mm_ps = ps0.tile([128, F], f32, space="PSUM", tag="p0mm")
                        for hh in range(2):
                            nc.tensor.matmul(
                                mm_ps[:rows, :], lhsT=xt_sb[:, hh, :rows],
                                rhs=w_t[:, hh, :], start=(hh == 0), stop=(hh == 1),
                            )
                        o_t = sb.tile([128, F], bf16, tag="p0o")
                        nc.vector.tensor_copy(out=o_t[:rows, :], in_=mm_ps[:rows, :])
                        nc.sync.dma_start(dram[row0:row0 + rows, :], o_t[:rows, :])

            # ---- P1: all-gather xl (two halves; A lands first) ----
            nc.gpsimd.collective_compute(
                "AllGather", mybir.AluOpType.bypass,
                replica_groups=[list(range(NCORES))],
                ins=[xl_sha.ap().opt()], outs=[xl_fa.ap().opt()],
            )
            nc.gpsimd.collective_compute(
                "AllGather", mybir.AluOpType.bypass,
                replica_groups=[list(range(NCORES))],
                ins=[xl_shb.ap().opt()], outs=[xl_fb.ap().opt()],
            )

            # ---- P2: paired-window edge processing ----
            sum_ps = accp.tile([128, 1], f32, space="PSUM", tag="accsum")
            ssq_ps = accp.tile([128, 1], f32, space="PSUM", tag="accssq")
            with tc.tile_pool(name="ps2", bufs=2, space="PSUM") as ps2:
                for wp in range(NP):
                    w0, w1 = 2 * wp, 2 * wp + 1
                    nL0, nL1 = int(cl[w0]), int(cl[w1])
                    nH0, nH1 = int(ch[w0]), int(ch[w1])
                    nLp = nL0 + nL1
                    nHp = nH0 + nH1
                    nTp = cpp[wp]
                    ix_t = sb.tile([128, cpp_max * 8], i16, tag="ix", bufs=3)
                    nc.sync.dma_start(
                        ix_t[:, :nTp * 8], idx_in[:, int(ioff[wp]):int(ioff[wp + 1])]
                    )
                    mk_t = sb.tile([128, cpp_max], f32, tag="mk", bufs=3)
                    nc.sync.dma_start(
                        mk_t[:, :nTp], mk_in[:, int(csum[wp]):int(csum[wp + 1])]
                    )
                    m01_t = sb.tile([128, cpp_max, W], bf16, tag="m01", bufs=3)
                    nc.sync.dma_start(
                        m01_t[:, :nTp, :],
                        m01_in[:, int(csum[wp]) * W:int(csum[wp + 1]) * W],
                    )
                    m01t_t = sb.tile([128, cpp_max, CHUNK], bf16, tag="m01t", bufs=3)
                    nc.sync.dma_start(
                        m01t_t[:, :nTp, :],
                        m01t_in[:, int(csum[wp]) * CHUNK:int(csum[wp + 1]) * CHUNK],
                    )
                    a_t = sb.tile([128, cpp_max, F], bf16, tag="a", bufs=3)
                    if nLp:
                        nc.gpsimd.dma_gather(
                            a_t[:, :nLp, :], xl_fa[:, :], ix_t[:, :nLp * 8],
                            nLp * CHUNK, nLp * CHUNK, F,
                            single_packet=(nLp * CHUNK <= 1024),
                        )
                    if nHp:
                        nc.gpsimd.dma_gather(
                            a_t[:, nLp:nTp, :], xl_fb[:, :],
                            ix_t[:, nLp * 8:nTp * 8], nHp * CHUNK, nHp * CHUNK, F,
                            single_packet=(nHp * CHUNK <= 1024),
                        )
                    for wi, w in ((0, w0), (1, w1)):
                        # chunk ranges of window w within the pair layout
                        ranges = []
                        if wi == 0:
                            if nL0:
                                ranges.append((0, nL0))
                            if nH0:
                                ranges.append((nLp, nLp + nH0))
                        else:
                            if nL1:
                                ranges.append((nL0, nLp))
                            if nH1:
                                ranges.append((nLp + nH0, nTp))
                        xrq_t = sb.tile([128, F], bf16, tag="xrq", bufs=3)
                        nc.sync.dma_start(xrq_t[:W, :], xr_d[w * W:(w + 1) * W, :])
                        # self-loop xl rows (local shard, contiguous)
                        a_self = sb.tile([128, F], bf16, tag="aself", bufs=2)
                        if wp == 0:
                            # init both ring buffers once; rows W:128 stay 0
                            nc.vector.memset(a_self[:], 0.0)
                        r0 = w * W
                        if r0 + W <= SPL:
                            nc.sync.dma_start(a_self[:W, :], xl_sha[r0:r0 + W, :])
                        elif r0 >= SPL:
                            nc.sync.dma_start(
                                a_self[:W, :], xl_shb[r0 - SPL:r0 - SPL + W, :]
                            )
                        else:
                            na = SPL - r0
                            nc.sync.dma_start(a_self[:na, :], xl_sha[r0:SPL, :])
                            nc.sync.dma_start(
                                a_self[na:W, :], xl_shb[0:W - na, :]
                            )
                        out_ps = ps2.tile([W, FQ], f32, space="PSUM", tag="outp")
                        # --- gathered chunks (self-loop chunk appended last) ---
                        nch = sum(e - b for b, e in ranges)
                        done = 0
                        for rb, re in ranges:
                            for k0 in range(rb, re, GRP):
                                g = min(GRP, re - k0)
                                l4 = sb.tile([128, GRP, F], bf16, tag="l4")
                                ap4 = sb.tile([128, GRP, FQ], bf16, tag="ap4")
                                for j in range(g):
                                    k = k0 + j
                                    b_ps = ps2.tile(
                                        [128, F], f32, space="PSUM", tag="bps", bufs=3
                                    )
                                    nc.tensor.matmul(
                                        b_ps[:], lhsT=m01t_t[:W, k, :], rhs=xrq_t[:W, :],
                                        start=True, stop=False,
                                    )
                                    nc.tensor.matmul(
                                        b_ps[:], lhsT=identb[:], rhs=a_t[:, k, :],
                                        start=False, stop=True,
                                    )
                                    nc.scalar.activation(
                                        l4[:, j, :], b_ps[:],
                                        mybir.ActivationFunctionType.Prelu, alpha=NEG,
                                    )
                                t4 = sb.tile([128, GRP, F], bf16, tag="t4")
                                nc.vector.tensor_tensor(
                                    out=t4[:, :g, :], in0=l4[:, :g, :],
                                    in1=attg[:, :g, :], op=mybir.AluOpType.mult,
                                )
                                lg4 = sb.tile([128, GRP, H], f32, tag="lg4")
                                nc.vector.reduce_sum(
                                    out=lg4[:, :g, :],
                                    in_=t4[:, :g, :].rearrange("p k (h c) -> p k h c", h=H),
                                    axis=mybir.AxisListType.X,
                                )
                                lgm4 = sb.tile([128, GRP, H], f32, tag="lgm4")
                                nc.vector.tensor_tensor(
                                    out=lgm4[:, :g, :], in0=lg4[:, :g, :],
                                    in1=mk_t[:, k0:k0 + g].rearrange(
                                        "p (g o) -> p g o", o=1
                                    ).to_broadcast([128, g, H]),
                                    op=mybir.AluOpType.add,
                                )
                                nc.scalar.activation(
                                    ap4[:, :g, F:FQ], lgm4[:, :g, :],
                                    mybir.ActivationFunctionType.Exp,
                                )
                                nc.vector.tensor_tensor(
                                    out=ap4[:, :g, 0:F].rearrange("p k (h c) -> p k h c", h=H),
                                    in0=a_t[:, k0:k0 + g, :].rearrange("p k (h c) -> p k h c", h=H),
                                    in1=ap4[:, :g, F:FQ].rearrange("p k (h o) -> p k h o", o=1).to_broadcast([128, g, H, C]),
                                    op=mybir.AluOpType.mult,
                                )
                                for j in range(g):
                                    k = k0 + j
                                    done += 1
                                    nc.tensor.matmul(
                                        out_ps[:], lhsT=m01_t[:, k, :], rhs=ap4[:, j, :],
                                        start=(done == 1), stop=False,
                                    )
                        # --- self-loop chunk (identity one-hots), last ---
                        bs_ps = ps2.tile([128, F], f32, space="PSUM", tag="bps", bufs=3)
                        nc.tensor.matmul(
                            bs_ps[:], lhsT=identb[:W, :], rhs=xrq_t[:W, :],
                            start=True, stop=False,
                        )
                        nc.tensor.matmul(
                            bs_ps[:], lhsT=identb[:], rhs=a_self[:],
                            start=False, stop=True,
                        )
                        l_s = sb.tile([128, F], bf16, tag="ls", bufs=2)
                        nc.scalar.activation(
                            l_s[:], bs_ps[:],
                            mybir.ActivationFunctionType.Prelu, alpha=NEG,
                        )
                        t_s = sb.tile([128, F], bf16, tag="ts", bufs=2)
                        nc.vector.tensor_tensor(
                            out=t_s[:], in0=l_s[:], in1=attg[:, 0, :],
                            op=mybir.AluOpType.mult,
                        )
                        lg_s = sb.tile([128, H], f32, tag="lgs", bufs=2)
                        nc.vector.reduce_sum(
                            out=lg_s[:],
                            in_=t_s[:].rearrange("p (h c) -> p h c", h=H),
                            axis=mybir.AxisListType.X,
                        )
                        ap_s = sb.tile([128, FQ], bf16, tag="aps", bufs=2)
                        nc.scalar.activation(
                            ap_s[:, F:FQ], lg_s[:],
                            mybir.ActivationFunctionType.Exp,
                            bias=mk_self[:], scale=1.0,
                        )
                        nc.vector.tensor_tensor(
                            out=ap_s[:, 0:F].rearrange("p (h c) -> p h c", h=H),
                            in0=a_self[:].rearrange("p (h c) -> p h c", h=H),
                            in1=ap_s[:, F:FQ].rearrange("p (h o) -> p h o", o=1).to_broadcast([128, H, C]),
                            op=mybir.AluOpType.mult,
                        )
                        nc.tensor.matmul(
                            out_ps[:], lhsT=identb[:, :W], rhs=ap_s[:],
                            start=False, stop=True,
                        )
                        # window epilogue
                        rd_t = sb.tile([128, H], f32, tag="rd")
                        nc.vector.reciprocal(rd_t[:W, :], out_ps[:, F:FQ])
                        oh_t = sb.tile([128, H, C], f32, tag="oh")
                        for h in range(H):
                            nc.vector.tensor_scalar(
                                out=oh_t[:W, h, :], in0=out_ps[:, h * C:(h + 1) * C],
                                scalar1=rd_t[:W, h:h + 1], scalar2=0.25,
                                op0=mybir.AluOpType.mult, op1=mybir.AluOpType.mult,
                            )
                        o01 = sb.tile([128, C], f32, tag="o01")
                        nc.vector.tensor_tensor(
                            out=o01[:W, :], in0=oh_t[:W, 0, :], in1=oh_t[:W, 1, :],
                            op=mybir.AluOpType.add,
                        )
                        o23 = sb.tile([128, C], f32, tag="o23")
                        nc.vector.tensor_tensor(
                            out=o23[:W, :], in0=oh_t[:W, 2, :], in1=oh_t[:W, 3, :],
                            op=mybir.AluOpType.add,
                        )
                        o0123 = sb.tile([128, C], f32, tag="o0123")
                        nc.vector.tensor_tensor(
                            out=o0123[:W, :], in0=o01[:W, :], in1=o23[:W, :],
                            op=mybir.AluOpType.add,
                        )
                        nc.vector.tensor_tensor(
                            out=om_all[:W, w, :], in0=o0123[:W, :], in1=biasb_t[:W, :],
                            op=mybir.AluOpType.add,
                        )
                        sq_t = sb.tile([128, C], f32, tag="sq")
                        nc.scalar.square(sq_t[:W, :], om_all[:W, w, :])
                        nc.tensor.matmul(
                            sum_ps[:C, :], lhsT=om_all[:W, w, :], rhs=ones_n[:W, :],
                            start=(w == 0), stop=(w == NW - 1),
                        )
                        nc.tensor.matmul(
                            ssq_ps[:C, :], lhsT=sq_t[:W, :], rhs=ones_n[:W, :],
                            start=(w == 0), stop=(w == NW - 1),
                        )

            # ---- P3: GraphNorm statistics ----
            st_t = sb.tile([128, 2], f32, tag="st")
            nc.vector.tensor_copy(out=st_t[:C, 0:1], in_=sum_ps[:C, :])
            nc.vector.tensor_copy(out=st_t[:C, 1:2], in_=ssq_ps[:C, :])
            nc.sync.dma_start(stats_l[:, :], st_t[:C, :])
            nc.gpsimd.collective_compute(
                "AllReduce", mybir.AluOpType.add,
                replica_groups=[list(range(NCORES))],
                ins=[stats_l.ap().opt()], outs=[stats_g.ap().opt()],
            )
            sg_t = sb.tile([128, 2], f32, tag="sg")
            nc.sync.dma_start(sg_t[:C, :], stats_g[:, :])
            mu_t = sb.tile([128, 1], f32, tag="mu")
            nc.vector.tensor_scalar_mul(mu_t[:C, :], sg_t[:C, 0:1], 1.0 / N)
            msq_t = sb.tile([128, 1], f32, tag="msq")
            nc.vector.tensor_scalar_mul(msq_t[:C, :], sg_t[:C, 1:2], 1.0 / N)
            amu_t = sb.tile([128, 1], f32, tag="amu")
            nc.vector.tensor_tensor(
                out=amu_t[:C, :], in0=gnp_t[:C, 2:3], in1=mu_t[:C, :],
                op=mybir.AluOpType.mult,
            )
            am2_t = sb.tile([128, 1], f32, tag="am2")
            nc.vector.scalar_tensor_tensor(
                out=am2_t[:C, :], in0=mu_t[:C, :], scalar=-2.0, in1=amu_t[:C, :],
                op0=mybir.AluOpType.mult, op1=mybir.AluOpType.add,
            )
            var_t = sb.tile([128, 1], f32, tag="var")
            nc.vector.tensor_tensor(
                out=var_t[:C, :], in0=amu_t[:C, :], in1=am2_t[:C, :],
                op=mybir.AluOpType.mult,
            )
            nc.vector.tensor_tensor(
                out=var_t[:C, :], in0=var_t[:C, :], in1=msq_t[:C, :],
                op=mybir.AluOpType.add,
            )
            nc.vector.tensor_scalar_add(var_t[:C, :], var_t[:C, :], EPS)
            sd_t = sb.tile([128, 1], f32, tag="sd")
            nc.scalar.sqrt(sd_t[:C, :], var_t[:C, :])
            inv_t = sb.tile([128, 1], f32, tag="inv")
            nc.vector.reciprocal(inv_t[:C, :], sd_t[:C, :])
            st2_t = sb.tile([128, 2], f32, tag="st2")
            nc.vector.tensor_tensor(
                out=st2_t[:C, 0:1], in0=gnp_t[:C, 0:1], in1=inv_t[:C, :],
                op=mybir.AluOpType.mult,
            )
            u_t = sb.tile([128, 1], f32, tag="u")
            nc.vector.tensor_tensor(
                out=u_t[:C, :], in0=st2_t[:C, 0:1], in1=amu_t[:C, :],
                op=mybir.AluOpType.mult,
            )
            nc.vector.tensor_tensor(
                out=st2_t[:C, 1:2], in0=gnp_t[:C, 1:2], in1=u_t[:C, :],
                op=mybir.AluOpType.subtract,
            )
            with tc.tile_pool(name="ps3", bufs=1, space="PSUM") as ps3:
                srow_ps = ps3.tile([128, C], f32, space="PSUM", tag="srow")
                nc.tensor.transpose(srow_ps[0:1, :], st2_t[:C, 0:1], ident[:C, :C])
                trow_ps = ps3.tile([128, C], f32, space="PSUM", tag="trow")
                nc.tensor.transpose(trow_ps[0:1, :], st2_t[:C, 1:2], ident[:C, :C])
                srow_t = sb.tile([128, C], f32, tag="srow")
                nc.vector.tensor_copy(out=srow_t[0:1, :], in_=srow_ps[0:1, :])
                trow_t = sb.tile([128, C], f32, tag="trow")
                nc.vector.tensor_copy(out=trow_t[0:1, :], in_=trow_ps[0:1, :])
                sb_ps = ps3.tile([128, C], f32, space="PSUM", tag="sbp")
                nc.tensor.matmul(
                    sb_ps[:W, :], lhsT=ones_r[0:1, :], rhs=srow_t[0:1, :],
                    start=True, stop=True,
                )
                nc.vector.tensor_copy(out=sb_t[:W, :], in_=sb_ps[:W, :])
                tb_ps = ps3.tile([128, C], f32, space="PSUM", tag="tbp")
                nc.tensor.matmul(
                    tb_ps[:W, :], lhsT=ones_r[0:1, :], rhs=trow_t[0:1, :],
                    start=True, stop=True,
                )
                nc.vector.tensor_copy(out=tb_t[:W, :], in_=tb_ps[:W, :])

            # ---- P4: apply GraphNorm affine (batched) ----
            y_all = sb.tile([128, NW, C], f32, tag="yall", bufs=1)
            nc.vector.tensor_tensor(
                out=y_all[:W, :, :], in0=om_all[:W, :, :],
                in1=sb_t[:W, :].rearrange("p (o c) -> p o c", o=1).to_broadcast([W, NW, C]),
                op=mybir.AluOpType.mult,
            )
            nc.vector.tensor_tensor(
                out=y_all[:W, :, :], in0=y_all[:W, :, :],
                in1=tb_t[:W, :].rearrange("p (o c) -> p o c", o=1).to_broadcast([W, NW, C]),
                op=mybir.AluOpType.add,
            )
            nc.sync.dma_start(
                out_ext.ap().rearrange("(w p) c -> p w c", p=W),
                om_all[:W, :, :] if dbg_om else y_all[:W, :, :],
            )
    nc.compile()
    return nc


def kernel(X, E, Wl, Wr, att, bias, gn_weight, gn_bias, gn_mean_scale, **kw):
    global LAST_RESULTS
    X = np.asarray(X, np.float32)
    E = np.asarray(E)
    Wl = np.asarray(Wl, np.float32)
    Wr = np.asarray(Wr, np.float32)
    att = np.asarray(att, np.float32)
    bias = np.asarray(bias, np.float32)
    gn_weight = np.asarray(gn_weight, np.float32)
    gn_bias = np.asarray(gn_bias, np.float32)
    gn_mean_scale = np.asarray(gn_mean_scale, np.float32)

    # self-loops handled separately on-device; only random edges here
    src = np.asarray(E[0], np.int64)
    dst = np.asarray(E[1], np.int64)
    cl, ch, per_core = _prep_edges(src, dst)

    attb = np.tile(att.reshape(1, F), (128, 1)).astype(np.float32)
    gnp = np.stack([gn_weight, gn_bias, gn_mean_scale, bias], axis=1).astype(np.float32)

    p0 = per_core[0]
    nc = _build(cl, ch, p0["IDX"].shape[1], p0["MK"].shape[1],
                p0["M01"].shape[1], p0["M01T"].shape[1])

    in_maps = []
    for c in range(NCORES):
        in_maps.append(
            dict(
                XS=np.ascontiguousarray(X[c * NSH:(c + 1) * NSH]),
                WL=Wl, WR=Wr, ATTB=attb, GNP=gnp,
                IDX=per_core[c]["IDX"], MK=per_core[c]["MK"],
                M01=per_core[c]["M01"], M01T=per_core[c]["M01T"],
            )
        )
    trace = bool(kw.get("trace"))
    res = run_bass_kernel_spmd(
        nc, in_maps, core_ids=list(range(NCORES)), trace=trace
    )
    LAST_RESULTS = res
    return np.concatenate([res.results[c]["OUT"] for c in range(NCORES)], axis=0)

